# revision 3
# baseline (speedup 1.0000x reference)
"""BotSpot GNN message-passing kernel for 8 TRN2 NeuronCores (Bass/Tile).

Strategy (data-parallel over the 8192-edge minibatch, 1024 edges/core):
  - host re-encodes the 1M-row device table into a [1M, 128] bf16
    full-feature table (cont + 7 embedding lookups per row) and the
    100K-row combin table into [100K, 128] bf16 (30 cont + channel emb)
  - device gathers one 256B row per neighbor incidence via indirect DMA
    (128 rows / instruction, 800 instructions per core == the SWDGE
    descriptor-generation floor of ~1 descriptor per incidence)
  - XBAR DMA transpose (HWDGE engines) turns gathered [incidence, feat]
    tiles into matmul-ready [feat, incidence] tiles — no PE transposes,
    no PSUM->SBUF copies
  - W_msg matmul + ReLU + positional segmented mean over each edge's 100
    neighbors; small per-edge MLP branches (channel, device, fusion,
    head) on-chip
"""

import numpy as np
import ml_dtypes

EMBED = 16
N_COMBIN, N_DEV, B, NB = 100000, 1000000, 8192, 100
DEV_CAPS = [50, 5, 30, 200, 500, 2000, 100]
D_DEV = 113
D_COMB = 46
D_DEV1, D_DEV2 = 67, 50
D_CH, D_MSG, D_FUS = 27, 67, 56
CAT_IN, D_C1, D_C2 = 106, 63, 31

N_CORES = 8
E_PER = B // N_CORES            # 1024 edges per core
INC_PER = E_PER * NB            # 102400 neighbor incidences per core
G_PER = INC_PER // 128          # 800 gathers per core
CHUNK_E = 32                    # edges per compute chunk
CHUNK_I = CHUNK_E * NB          # 3200 incidences per chunk
G_CHUNK = CHUNK_I // 128        # 25 gathers per chunk
N_CHUNK = E_PER // CHUNK_E      # 32 chunks


def _wrap_clamp_np(i, n):
    """jnp.ndarray[idx] semantics: negative wraps once, then clamp."""
    i = np.where(i < 0, i + n, i)
    return np.clip(i, 0, n - 1)


def _build_xfull(device_feats, tabs):
    """[1M, 128] bf16: col0 cont, cols 1:113 the 7 embeddings in
    reference order (lang, plat, os, country, carrier, brand, plat_os)."""
    out = np.zeros((N_DEV, 128), ml_dtypes.bfloat16)
    out[:, 0] = device_feats[:, 0].astype(ml_dtypes.bfloat16)
    cats = device_feats[:, 1:8].astype(np.int32)
    for i, (cap, t) in enumerate(zip(DEV_CAPS, tabs)):
        c = _wrap_clamp_np(cats[:, i], cap)
        out[:, 1 + 16 * i:17 + 16 * i] = t[c].astype(ml_dtypes.bfloat16)
    return out


def _build_comb_ext(combin_feats, channel_id_emb):
    """[100K, 128] bf16: cols 0:30 cont, 30:46 channel emb, rest 0."""
    out = np.zeros((N_COMBIN, 128), ml_dtypes.bfloat16)
    out[:, :30] = combin_feats[:, :30].astype(ml_dtypes.bfloat16)
    cid = _wrap_clamp_np(combin_feats[:, 30].astype(np.int32), N_COMBIN)
    out[:, 30:46] = channel_id_emb[cid].astype(ml_dtypes.bfloat16)
    return out


def _run(inputs, trace=False):
    import concourse.bass as bass
    import concourse.bacc as bacc
    import concourse.mybir as mybir
    import concourse.tile as tile
    from concourse.bass_utils import run_bass_kernel_spmd

    f32, bf16, i32 = mybir.dt.float32, mybir.dt.bfloat16, mybir.dt.int32

    combin_feats = np.asarray(inputs["combin_feats"], np.float32)
    device_feats = np.asarray(inputs["device_feats"], np.float32)
    channel_id_emb = np.asarray(inputs["channel_id_emb"], np.float32)
    tabs = [np.asarray(inputs[k], np.float32) for k in
            ("lang_emb", "plat_emb", "os_emb", "country_emb",
             "carrier_emb", "brand_emb", "plat_os_emb")]
    edges = np.asarray(inputs["edges"], np.int64)
    neibrs = np.asarray(inputs["sampled_neibrs"], np.int64)

    xfull = _build_xfull(device_feats, tabs)
    comb_ext = _build_comb_ext(combin_feats, channel_id_emb)

    def W(name):
        return np.asarray(inputs[name], np.float32)

    def lhsT_pad(w, kpad):  # [out,in] f32 -> [kpad, out] bf16
        t = np.zeros((kpad, w.shape[0]), np.float32)
        t[: w.shape[1], :] = w.T
        return t.astype(ml_dtypes.bfloat16)

    Wm_l = lhsT_pad(W("W_msg"), 128)                         # [128, 67]
    Wd1_l = lhsT_pad(W("W_dev1"), 128)                       # [128, 67]
    Wch_l = lhsT_pad(W("W_ch1"), 128)                        # [128, 27]
    Wd2_l = lhsT_pad(W("W_dev2"), 67)                        # [67, 50]
    Wfc_l = lhsT_pad(W("W_fus")[:, :D_CH], 27)               # [27, 56]
    Wfm_l = lhsT_pad(W("W_fus")[:, D_CH:] / NB, 67)          # [67, 56]
    Wc1f_l = lhsT_pad(W("W_c1")[:, :D_FUS], 56)              # [56, 63]
    Wc1d_l = lhsT_pad(W("W_c1")[:, D_FUS:], 50)              # [50, 63]
    Wc2_l = lhsT_pad(W("W_c2"), 63)                          # [63, 31]
    Wc3_l = lhsT_pad(W("W_c3"), 31)                          # [31, 1]

    biases = np.zeros((128, 8), np.float32)
    for j, nm in enumerate(("b_msg", "b_dev1", "b_ch1", "b_dev2",
                            "b_fus", "b_c1", "b_c2", "b_c3")):
        b = W(nm)
        biases[: len(b), j] = b

    # ---- host index prep (per core) ----
    e_comb = _wrap_clamp_np(edges[:, 0], N_COMBIN).astype(np.int32)
    e_dev = _wrap_clamp_np(edges[:, 1], N_DEV).astype(np.int32)
    nb_idx = _wrap_clamp_np(neibrs, N_DEV).astype(np.int32)  # [B, 100]

    nbr_idx_np = np.zeros((N_CORES, 128, G_PER), np.int32)
    for c in range(N_CORES):
        flat = nb_idx[c * E_PER:(c + 1) * E_PER].reshape(-1)  # [102400]
        nbr_idx_np[c] = flat.reshape(G_PER, 128).T

    def edge_idx_arr(v):
        out = np.zeros((N_CORES, 128, 8), np.int32)
        for c in range(N_CORES):
            out[c] = v[c * E_PER:(c + 1) * E_PER].reshape(8, 128).T
        return out

    comb_idx_np = edge_idx_arr(e_comb)
    dev_idx_np = edge_idx_arr(e_dev)

    # ---- build bass kernel ----
    nc = bacc.Bacc("TRN2", target_bir_lowering=False, debug=False,
                   num_devices=N_CORES)

    def dram(name, arr, dtype):
        return nc.dram_tensor(name, list(arr.shape), dtype,
                              kind="ExternalInput").ap()

    xf_t = dram("xf_t", xfull, bf16)
    ce_t = dram("ce_t", comb_ext, bf16)
    nbr_t = dram("nbr_t", nbr_idx_np[0], i32)
    ci_t = dram("ci_t", comb_idx_np[0], i32)
    di_t = dram("di_t", dev_idx_np[0], i32)
    wm_t = dram("wm_t", Wm_l, bf16)
    wd1_t = dram("wd1_t", Wd1_l, bf16)
    wch_t = dram("wch_t", Wch_l, bf16)
    wd2_t = dram("wd2_t", Wd2_l, bf16)
    wfc_t = dram("wfc_t", Wfc_l, bf16)
    wfm_t = dram("wfm_t", Wfm_l, bf16)
    wc1f_t = dram("wc1f_t", Wc1f_l, bf16)
    wc1d_t = dram("wc1d_t", Wc1d_l, bf16)
    wc2_t = dram("wc2_t", Wc2_l, bf16)
    wc3_t = dram("wc3_t", Wc3_l, bf16)
    bias_t = dram("bias_t", biases, f32)
    out_t = nc.dram_tensor("out", [1, E_PER], f32, kind="ExternalOutput").ap()

    IOA = bass.IndirectOffsetOnAxis
    AX = mybir.AxisListType
    ALU = mybir.AluOpType
    ACTF = mybir.ActivationFunctionType

    with tile.TileContext(nc, trace_sim=False) as tc:
        with tc.tile_pool(name="const", bufs=1) as cpool, \
             tc.tile_pool(name="sbuf", bufs=2) as pool, \
             tc.tile_pool(name="xg", bufs=32) as gpool, \
             tc.tile_pool(name="xt", bufs=3) as xtpool, \
             tc.tile_pool(name="big", bufs=1) as bigpool, \
             tc.tile_pool(name="psum", bufs=4, space="PSUM") as pp:

            def cload(name, t, shape, dtype):
                s = cpool.tile(shape, dtype, tag=name)
                nc.sync.dma_start(out=s[:], in_=t[:])
                return s

            wm = cload("wm", wm_t, [128, 67], bf16)
            wd1 = cload("wd1", wd1_t, [128, 67], bf16)
            wch = cload("wch", wch_t, [128, 27], bf16)
            wd2 = cload("wd2", wd2_t, [67, 50], bf16)
            wfc = cload("wfc", wfc_t, [27, 56], bf16)
            wfm = cload("wfm", wfm_t, [67, 56], bf16)
            wc1f = cload("wc1f", wc1f_t, [56, 63], bf16)
            wc1d = cload("wc1d", wc1d_t, [50, 63], bf16)
            wc2 = cload("wc2", wc2_t, [63, 31], bf16)
            wc3 = cload("wc3", wc3_t, [31, 1], bf16)
            bias = cload("bias", bias_t, [128, 8], f32)
            nbr_i = cload("nbr", nbr_t, [128, G_PER], i32)
            ci = cload("ci", ci_t, [128, 8], i32)
            di = cload("di", di_t, [128, 8], i32)

            msg = bigpool.tile([67, E_PER], f32)
            xdt = bigpool.tile([128, E_PER], bf16)
            xct = bigpool.tile([128, E_PER], bf16)

            def gather_T(dst, table, idx_col, k):
                """gather 128 rows of `table` -> transpose into dst cols."""
                xg = gpool.tile([128, 128], bf16, tag="xg")
                nc.gpsimd.indirect_dma_start(
                    out=xg[:], out_offset=None, in_=table[:],
                    in_offset=IOA(ap=idx_col, axis=0))
                eng = nc.sync if k % 2 == 0 else nc.scalar
                eng.dma_start(out=dst, in_=xg[:], transpose=True)

            # ---- edge-branch gathers (16 Pool instrs) ----
            for k in range(8):
                gather_T(xdt[:, k * 128:(k + 1) * 128], xf_t,
                         di[:, k:k + 1], k)
            for k in range(8):
                gather_T(xct[:, k * 128:(k + 1) * 128], ce_t,
                         ci[:, k:k + 1], k)

            # ---- edge MLP part 1: d1, d2, ch ----
            d1 = bigpool.tile([67, E_PER], bf16)
            d2 = bigpool.tile([50, E_PER], bf16)
            ch = bigpool.tile([27, E_PER], bf16)
            for half in range(2):
                sl = slice(half * 512, half * 512 + 512)
                p1 = pp.tile([67, 512], f32, tag="ep", space="PSUM")
                nc.tensor.matmul(out=p1[:], lhsT=wd1[:], rhs=xdt[:, sl],
                                 start=True, stop=True)
                nc.scalar.activation(out=d1[:, sl], in_=p1[:], func=ACTF.Relu,
                                     bias=bias[:67, 1:2], scale=1.0)
                p2 = pp.tile([50, 512], f32, tag="ep", space="PSUM")
                nc.tensor.matmul(out=p2[:], lhsT=wd2[:], rhs=d1[:67, sl],
                                 start=True, stop=True)
                nc.scalar.activation(out=d2[:, sl], in_=p2[:], func=ACTF.Relu,
                                     bias=bias[:50, 3:4], scale=1.0)
                p3 = pp.tile([27, 512], f32, tag="ep", space="PSUM")
                nc.tensor.matmul(out=p3[:], lhsT=wch[:], rhs=xct[:, sl],
                                 start=True, stop=True)
                nc.scalar.activation(out=ch[:, sl], in_=p3[:], func=ACTF.Relu,
                                     bias=bias[:27, 2:3], scale=1.0)

            # ---- neighbor pipeline ----
            for c in range(N_CHUNK):
                xt = xtpool.tile([128, CHUNK_I], bf16, tag="xt")
                for t in range(G_CHUNK):
                    g = c * G_CHUNK + t
                    gather_T(xt[:, t * 128:(t + 1) * 128], xf_t,
                             nbr_i[:, g:g + 1], g)
                for k in range(8):
                    p = pp.tile([67, 400], f32, tag="mp", space="PSUM")
                    nc.tensor.matmul(out=p[:], lhsT=wm[:],
                                     rhs=xt[:, k * 400:(k + 1) * 400],
                                     start=True, stop=True)
                    rr = pool.tile([67, 400], bf16, tag="rr")
                    nc.scalar.activation(out=rr[:], in_=p[:], func=ACTF.Relu,
                                         bias=bias[:67, 0:1], scale=1.0)
                    e0 = c * CHUNK_E + k * 4
                    nc.vector.tensor_reduce(
                        out=msg[:, e0:e0 + 4],
                        in_=rr[:].rearrange("p (e k) -> p e k", k=NB),
                        axis=AX.X, op=ALU.add)

            # ---- edge MLP part 2: fus, head ----
            msgb = bigpool.tile([67, E_PER], bf16)
            nc.vector.tensor_copy(out=msgb[:], in_=msg[:])
            fus = bigpool.tile([56, E_PER], bf16)
            h1 = bigpool.tile([63, E_PER], bf16)
            h2 = bigpool.tile([31, E_PER], bf16)
            hout = bigpool.tile([1, E_PER], f32)
            for half in range(2):
                sl = slice(half * 512, half * 512 + 512)
                p4 = pp.tile([56, 512], f32, tag="ep", space="PSUM")
                nc.tensor.matmul(out=p4[:], lhsT=wfc[:], rhs=ch[:27, sl],
                                 start=True, stop=False)
                nc.tensor.matmul(out=p4[:], lhsT=wfm[:], rhs=msgb[:67, sl],
                                 start=False, stop=True)
                nc.scalar.activation(out=fus[:, sl], in_=p4[:], func=ACTF.Relu,
                                     bias=bias[:56, 4:5], scale=1.0)
                p5 = pp.tile([63, 512], f32, tag="ep", space="PSUM")
                nc.tensor.matmul(out=p5[:], lhsT=wc1f[:], rhs=fus[:56, sl],
                                 start=True, stop=False)
                nc.tensor.matmul(out=p5[:], lhsT=wc1d[:], rhs=d2[:50, sl],
                                 start=False, stop=True)
                nc.scalar.activation(out=h1[:, sl], in_=p5[:], func=ACTF.Relu,
                                     bias=bias[:63, 5:6], scale=1.0)
                p6 = pp.tile([31, 512], f32, tag="ep", space="PSUM")
                nc.tensor.matmul(out=p6[:], lhsT=wc2[:], rhs=h1[:63, sl],
                                 start=True, stop=True)
                nc.scalar.activation(out=h2[:, sl], in_=p6[:], func=ACTF.Relu,
                                     bias=bias[:31, 6:7], scale=1.0)
                p7 = pp.tile([1, 512], f32, tag="ep", space="PSUM")
                nc.tensor.matmul(out=p7[:], lhsT=wc3[:], rhs=h2[:31, sl],
                                 start=True, stop=True)
                nc.scalar.activation(out=hout[:, sl], in_=p7[:],
                                     func=ACTF.Identity, bias=bias[:1, 7:8],
                                     scale=1.0)
            nc.sync.dma_start(out=out_t[:], in_=hout[:])

    nc.compile()

    base = {
        "xf_t": xfull, "ce_t": comb_ext,
        "wm_t": Wm_l, "wd1_t": Wd1_l, "wch_t": Wch_l, "wd2_t": Wd2_l,
        "wfc_t": Wfc_l, "wfm_t": Wfm_l, "wc1f_t": Wc1f_l,
        "wc1d_t": Wc1d_l, "wc2_t": Wc2_l, "wc3_t": Wc3_l, "bias_t": biases,
    }
    in_maps = []
    for c in range(N_CORES):
        m = dict(base)
        m["nbr_t"] = nbr_idx_np[c]
        m["ci_t"] = comb_idx_np[c]
        m["di_t"] = dev_idx_np[c]
        in_maps.append(m)

    res = run_bass_kernel_spmd(nc, in_maps, core_ids=list(range(N_CORES)),
                               trace=trace)
    outs = [res.results[c]["out"].reshape(E_PER) for c in range(N_CORES)]
    full = np.concatenate(outs).reshape(B, 1).astype(np.float32)
    return full, res


def kernel(**inputs):
    out, _ = _run(inputs, trace=False)
    return out


# revision 7
# speedup vs baseline: 3.9817x; 3.9817x over previous
"""BotSpot GNN message-passing kernel for 8 TRN2 NeuronCores (Bass/Tile).

Strategy (data-parallel over the 8192-edge minibatch, 1024 edges/core):
  - host re-encodes the 1M-row device table into a [1M, 128] bf16
    full-feature table (cont + 7 embedding lookups per row) and the
    100K-row combin table into [100K, 128] bf16 (30 cont + channel emb)
  - device gathers one 256B row per neighbor incidence via indirect DMA
    (128 rows / instruction, 800 instructions per core == the SWDGE
    descriptor-generation floor of ~1 descriptor per incidence)
  - XBAR DMA transpose (HWDGE engines) turns gathered [incidence, feat]
    tiles into matmul-ready [feat, incidence] tiles — no PE transposes,
    no PSUM->SBUF copies
  - W_msg matmul + ReLU + positional segmented mean over each edge's 100
    neighbors; small per-edge MLP branches (channel, device, fusion,
    head) on-chip
"""

import numpy as np
import ml_dtypes

EMBED = 16
N_COMBIN, N_DEV, B, NB = 100000, 1000000, 8192, 100
DEV_CAPS = [50, 5, 30, 200, 500, 2000, 100]
D_DEV = 113
D_COMB = 46
D_DEV1, D_DEV2 = 67, 50
D_CH, D_MSG, D_FUS = 27, 67, 56
CAT_IN, D_C1, D_C2 = 106, 63, 31

N_CORES = 8
E_PER = B // N_CORES            # 1024 edges per core
INC_PER = E_PER * NB            # 102400 neighbor incidences per core
G_PER = INC_PER // 128          # 800 gathers per core
CHUNK_E = 32                    # edges per compute chunk
CHUNK_I = CHUNK_E * NB          # 3200 incidences per chunk
G_CHUNK = CHUNK_I // 128        # 25 gathers per chunk
N_CHUNK = E_PER // CHUNK_E      # 32 chunks


def _wrap_clamp_np(i, n):
    """jnp.ndarray[idx] semantics: negative wraps once, then clamp."""
    i = np.where(i < 0, i + n, i)
    return np.clip(i, 0, n - 1)


def _build_xfull(device_feats, tabs):
    """[1M, 128] bf16: col0 cont, cols 1:113 the 7 embeddings in
    reference order (lang, plat, os, country, carrier, brand, plat_os)."""
    out = np.zeros((N_DEV, 128), ml_dtypes.bfloat16)
    out[:, 0] = device_feats[:, 0].astype(ml_dtypes.bfloat16)
    cats = device_feats[:, 1:8].astype(np.int32)
    for i, (cap, t) in enumerate(zip(DEV_CAPS, tabs)):
        c = _wrap_clamp_np(cats[:, i], cap)
        out[:, 1 + 16 * i:17 + 16 * i] = t[c].astype(ml_dtypes.bfloat16)
    return out


def _build_comb_ext(combin_feats, channel_id_emb):
    """[100K, 128] bf16: cols 0:30 cont, 30:46 channel emb, rest 0."""
    out = np.zeros((N_COMBIN, 128), ml_dtypes.bfloat16)
    out[:, :30] = combin_feats[:, :30].astype(ml_dtypes.bfloat16)
    cid = _wrap_clamp_np(combin_feats[:, 30].astype(np.int32), N_COMBIN)
    out[:, 30:46] = channel_id_emb[cid].astype(ml_dtypes.bfloat16)
    return out


def _run(inputs, trace=False):
    import concourse.bass as bass
    import concourse.bacc as bacc
    import concourse.mybir as mybir
    import concourse.tile as tile
    from concourse.bass_utils import run_bass_kernel_spmd

    f32, bf16, i32 = mybir.dt.float32, mybir.dt.bfloat16, mybir.dt.int32

    combin_feats = np.asarray(inputs["combin_feats"], np.float32)
    device_feats = np.asarray(inputs["device_feats"], np.float32)
    channel_id_emb = np.asarray(inputs["channel_id_emb"], np.float32)
    tabs = [np.asarray(inputs[k], np.float32) for k in
            ("lang_emb", "plat_emb", "os_emb", "country_emb",
             "carrier_emb", "brand_emb", "plat_os_emb")]
    edges = np.asarray(inputs["edges"], np.int64)
    neibrs = np.asarray(inputs["sampled_neibrs"], np.int64)

    xfull = _build_xfull(device_feats, tabs)
    comb_ext = _build_comb_ext(combin_feats, channel_id_emb)

    def W(name):
        return np.asarray(inputs[name], np.float32)

    def lhsT_pad(w, kpad):  # [out,in] f32 -> [kpad, out] bf16
        t = np.zeros((kpad, w.shape[0]), np.float32)
        t[: w.shape[1], :] = w.T
        return t.astype(ml_dtypes.bfloat16)

    Wm_l = lhsT_pad(W("W_msg"), 128)                         # [128, 67]
    Wd1_l = lhsT_pad(W("W_dev1"), 128)                       # [128, 67]
    Wch_l = lhsT_pad(W("W_ch1"), 128)                        # [128, 27]
    Wd2_l = lhsT_pad(W("W_dev2"), 67)                        # [67, 50]
    Wfc_l = lhsT_pad(W("W_fus")[:, :D_CH], 27)               # [27, 56]
    Wfm_l = lhsT_pad(W("W_fus")[:, D_CH:] / NB, 67)          # [67, 56]
    Wc1f_l = lhsT_pad(W("W_c1")[:, :D_FUS], 56)              # [56, 63]
    Wc1d_l = lhsT_pad(W("W_c1")[:, D_FUS:], 50)              # [50, 63]
    Wc2_l = lhsT_pad(W("W_c2"), 63)                          # [63, 31]
    Wc3_l = lhsT_pad(W("W_c3"), 31)                          # [31, 1]

    biases = np.zeros((128, 8), np.float32)
    for j, nm in enumerate(("b_msg", "b_dev1", "b_ch1", "b_dev2",
                            "b_fus", "b_c1", "b_c2", "b_c3")):
        b = W(nm)
        biases[: len(b), j] = b

    # ---- host index prep (per core) ----
    e_comb = _wrap_clamp_np(edges[:, 0], N_COMBIN).astype(np.int32)
    e_dev = _wrap_clamp_np(edges[:, 1], N_DEV).astype(np.int32)
    nb_idx = _wrap_clamp_np(neibrs, N_DEV).astype(np.int32)  # [B, 100]

    nbr_idx_np = np.zeros((N_CORES, 128, G_PER), np.int32)
    for c in range(N_CORES):
        flat = nb_idx[c * E_PER:(c + 1) * E_PER].reshape(-1)  # [102400]
        nbr_idx_np[c] = flat.reshape(G_PER, 128).T

    def edge_idx_arr(v):
        out = np.zeros((N_CORES, 128, 8), np.int32)
        for c in range(N_CORES):
            out[c] = v[c * E_PER:(c + 1) * E_PER].reshape(8, 128).T
        return out

    comb_idx_np = edge_idx_arr(e_comb)
    dev_idx_np = edge_idx_arr(e_dev)

    # ---- build bass kernel ----
    nc = bacc.Bacc("TRN2", target_bir_lowering=False, debug=False,
                   num_devices=N_CORES)

    def dram(name, arr, dtype):
        return nc.dram_tensor(name, list(arr.shape), dtype,
                              kind="ExternalInput").ap()

    xf_t = dram("xf_t", xfull, bf16)
    ce_t = dram("ce_t", comb_ext, bf16)
    nbr_t = dram("nbr_t", nbr_idx_np[0], i32)
    ci_t = dram("ci_t", comb_idx_np[0], i32)
    di_t = dram("di_t", dev_idx_np[0], i32)
    wm_t = dram("wm_t", Wm_l, bf16)
    wd1_t = dram("wd1_t", Wd1_l, bf16)
    wch_t = dram("wch_t", Wch_l, bf16)
    wd2_t = dram("wd2_t", Wd2_l, bf16)
    wfc_t = dram("wfc_t", Wfc_l, bf16)
    wfm_t = dram("wfm_t", Wfm_l, bf16)
    wc1f_t = dram("wc1f_t", Wc1f_l, bf16)
    wc1d_t = dram("wc1d_t", Wc1d_l, bf16)
    wc2_t = dram("wc2_t", Wc2_l, bf16)
    wc3_t = dram("wc3_t", Wc3_l, bf16)
    bias_t = dram("bias_t", biases, f32)
    out_t = nc.dram_tensor("out", [1, E_PER], f32, kind="ExternalOutput").ap()

    IOA = bass.IndirectOffsetOnAxis
    AX = mybir.AxisListType
    ALU = mybir.AluOpType
    ACTF = mybir.ActivationFunctionType

    from concourse.masks import make_identity

    with tile.TileContext(nc, trace_sim=False) as tc:
        with tc.tile_pool(name="const", bufs=1) as cpool, \
             tc.tile_pool(name="sbuf", bufs=2) as pool, \
             tc.tile_pool(name="xg", bufs=12) as gpool, \
             tc.tile_pool(name="xt", bufs=3) as xtpool, \
             tc.tile_pool(name="big", bufs=1) as bigpool, \
             tc.tile_pool(name="tp", bufs=4, space="PSUM") as tpp, \
             tc.tile_pool(name="psum", bufs=2, space="PSUM") as pp:

            def cload(name, t, shape, dtype):
                s = cpool.tile(shape, dtype, tag=name)
                nc.sync.dma_start(out=s[:], in_=t[:])
                return s

            wm = cload("wm", wm_t, [128, 67], bf16)
            wd1 = cload("wd1", wd1_t, [128, 67], bf16)
            wch = cload("wch", wch_t, [128, 27], bf16)
            wd2 = cload("wd2", wd2_t, [67, 50], bf16)
            wfc = cload("wfc", wfc_t, [27, 56], bf16)
            wfm = cload("wfm", wfm_t, [67, 56], bf16)
            wc1f = cload("wc1f", wc1f_t, [56, 63], bf16)
            wc1d = cload("wc1d", wc1d_t, [50, 63], bf16)
            wc2 = cload("wc2", wc2_t, [63, 31], bf16)
            wc3 = cload("wc3", wc3_t, [31, 1], bf16)
            bias = cload("bias", bias_t, [128, 8], f32)
            nbr_i = cload("nbr", nbr_t, [128, G_PER], i32)
            ci = cload("ci", ci_t, [128, 8], i32)
            di = cload("di", di_t, [128, 8], i32)

            ident = cpool.tile([128, 128], bf16)
            make_identity(nc, ident[:])

            msg = bigpool.tile([67, E_PER], f32)
            xdt = bigpool.tile([128, E_PER], bf16)
            xct = bigpool.tile([128, E_PER], bf16)

            def gather_T(dst, table, idx_col, k):
                """gather 128 rows of `table` -> PE transpose into dst cols."""
                xg = gpool.tile([128, 128], bf16, tag="xg")
                nc.gpsimd.indirect_dma_start(
                    out=xg[:], out_offset=None, in_=table[:],
                    in_offset=IOA(ap=idx_col, axis=0))
                tp = tpp.tile([128, 128], bf16, tag="tp", space="PSUM")
                nc.tensor.transpose(out=tp[:], in_=xg[:], identity=ident[:])
                if k % 2 == 0:
                    nc.scalar.copy(out=dst, in_=tp[:])
                else:
                    nc.vector.tensor_copy(out=dst, in_=tp[:])

            # ---- edge-branch gathers (16 Pool instrs) ----
            for k in range(8):
                gather_T(xdt[:, k * 128:(k + 1) * 128], xf_t,
                         di[:, k:k + 1], k)
            for k in range(8):
                gather_T(xct[:, k * 128:(k + 1) * 128], ce_t,
                         ci[:, k:k + 1], k)

            # ---- edge MLP part 1: d1, d2, ch ----
            d1 = bigpool.tile([67, E_PER], bf16)
            d2 = bigpool.tile([50, E_PER], bf16)
            ch = bigpool.tile([27, E_PER], bf16)
            for half in range(2):
                sl = slice(half * 512, half * 512 + 512)
                p1 = pp.tile([67, 512], f32, tag="ep", space="PSUM")
                nc.tensor.matmul(out=p1[:], lhsT=wd1[:], rhs=xdt[:, sl],
                                 start=True, stop=True)
                nc.scalar.activation(out=d1[:, sl], in_=p1[:], func=ACTF.Relu,
                                     bias=bias[:67, 1:2], scale=1.0)
                p2 = pp.tile([50, 512], f32, tag="ep", space="PSUM")
                nc.tensor.matmul(out=p2[:], lhsT=wd2[:], rhs=d1[:67, sl],
                                 start=True, stop=True)
                nc.scalar.activation(out=d2[:, sl], in_=p2[:], func=ACTF.Relu,
                                     bias=bias[:50, 3:4], scale=1.0)
                p3 = pp.tile([27, 512], f32, tag="ep", space="PSUM")
                nc.tensor.matmul(out=p3[:], lhsT=wch[:], rhs=xct[:, sl],
                                 start=True, stop=True)
                nc.scalar.activation(out=ch[:, sl], in_=p3[:], func=ACTF.Relu,
                                     bias=bias[:27, 2:3], scale=1.0)

            # ---- neighbor pipeline ----
            for c in range(N_CHUNK):
                xt = xtpool.tile([128, CHUNK_I], bf16, tag="xt")
                for t in range(G_CHUNK):
                    g = c * G_CHUNK + t
                    gather_T(xt[:, t * 128:(t + 1) * 128], xf_t,
                             nbr_i[:, g:g + 1], g)
                for k in range(8):
                    p = pp.tile([67, 400], f32, tag="mp", space="PSUM")
                    nc.tensor.matmul(out=p[:], lhsT=wm[:],
                                     rhs=xt[:, k * 400:(k + 1) * 400],
                                     start=True, stop=True)
                    rr = pool.tile([67, 400], bf16, tag="rr")
                    nc.scalar.activation(out=rr[:], in_=p[:], func=ACTF.Relu,
                                         bias=bias[:67, 0:1], scale=1.0)
                    e0 = c * CHUNK_E + k * 4
                    nc.vector.tensor_reduce(
                        out=msg[:, e0:e0 + 4],
                        in_=rr[:].rearrange("p (e k) -> p e k", k=NB),
                        axis=AX.X, op=ALU.add)

            # ---- edge MLP part 2: fus, head ----
            msgb = bigpool.tile([67, E_PER], bf16)
            nc.vector.tensor_copy(out=msgb[:], in_=msg[:])
            fus = bigpool.tile([56, E_PER], bf16)
            h1 = bigpool.tile([63, E_PER], bf16)
            h2 = bigpool.tile([31, E_PER], bf16)
            hout = bigpool.tile([1, E_PER], f32)
            for half in range(2):
                sl = slice(half * 512, half * 512 + 512)
                p4 = pp.tile([56, 512], f32, tag="ep", space="PSUM")
                nc.tensor.matmul(out=p4[:], lhsT=wfc[:], rhs=ch[:27, sl],
                                 start=True, stop=False)
                nc.tensor.matmul(out=p4[:], lhsT=wfm[:], rhs=msgb[:67, sl],
                                 start=False, stop=True)
                nc.scalar.activation(out=fus[:, sl], in_=p4[:], func=ACTF.Relu,
                                     bias=bias[:56, 4:5], scale=1.0)
                p5 = pp.tile([63, 512], f32, tag="ep", space="PSUM")
                nc.tensor.matmul(out=p5[:], lhsT=wc1f[:], rhs=fus[:56, sl],
                                 start=True, stop=False)
                nc.tensor.matmul(out=p5[:], lhsT=wc1d[:], rhs=d2[:50, sl],
                                 start=False, stop=True)
                nc.scalar.activation(out=h1[:, sl], in_=p5[:], func=ACTF.Relu,
                                     bias=bias[:63, 5:6], scale=1.0)
                p6 = pp.tile([31, 512], f32, tag="ep", space="PSUM")
                nc.tensor.matmul(out=p6[:], lhsT=wc2[:], rhs=h1[:63, sl],
                                 start=True, stop=True)
                nc.scalar.activation(out=h2[:, sl], in_=p6[:], func=ACTF.Relu,
                                     bias=bias[:31, 6:7], scale=1.0)
                p7 = pp.tile([1, 512], f32, tag="ep", space="PSUM")
                nc.tensor.matmul(out=p7[:], lhsT=wc3[:], rhs=h2[:31, sl],
                                 start=True, stop=True)
                nc.scalar.activation(out=hout[:, sl], in_=p7[:],
                                     func=ACTF.Identity, bias=bias[:1, 7:8],
                                     scale=1.0)
            nc.sync.dma_start(out=out_t[:], in_=hout[:])

    nc.compile()

    base = {
        "xf_t": xfull, "ce_t": comb_ext,
        "wm_t": Wm_l, "wd1_t": Wd1_l, "wch_t": Wch_l, "wd2_t": Wd2_l,
        "wfc_t": Wfc_l, "wfm_t": Wfm_l, "wc1f_t": Wc1f_l,
        "wc1d_t": Wc1d_l, "wc2_t": Wc2_l, "wc3_t": Wc3_l, "bias_t": biases,
    }
    in_maps = []
    for c in range(N_CORES):
        m = dict(base)
        m["nbr_t"] = nbr_idx_np[c]
        m["ci_t"] = comb_idx_np[c]
        m["di_t"] = dev_idx_np[c]
        in_maps.append(m)

    res = run_bass_kernel_spmd(nc, in_maps, core_ids=list(range(N_CORES)),
                               trace=trace)
    outs = [res.results[c]["out"].reshape(E_PER) for c in range(N_CORES)]
    full = np.concatenate(outs).reshape(B, 1).astype(np.float32)
    return full, res


def kernel(**inputs):
    out, _ = _run(inputs, trace=False)
    return out


# revision 10
# speedup vs baseline: 4.0266x; 1.0113x over previous
"""BotSpot GNN message-passing kernel for 8 TRN2 NeuronCores (Bass/Tile).

Strategy (data-parallel over the 8192-edge minibatch, 1024 edges/core):
  - host re-encodes the 1M-row device table into a [1M, 128] bf16
    full-feature table (cont + 7 embedding lookups per row) and the
    100K-row combin table into [100K, 128] bf16 (30 cont + channel emb)
  - device gathers one 256B row per neighbor incidence via indirect DMA
    (128 rows / instruction, 800 instructions per core == the SWDGE
    descriptor-generation floor of ~1 descriptor per incidence)
  - XBAR DMA transpose (HWDGE engines) turns gathered [incidence, feat]
    tiles into matmul-ready [feat, incidence] tiles — no PE transposes,
    no PSUM->SBUF copies
  - W_msg matmul + ReLU + positional segmented mean over each edge's 100
    neighbors; small per-edge MLP branches (channel, device, fusion,
    head) on-chip
"""

import numpy as np
import ml_dtypes

EMBED = 16
N_COMBIN, N_DEV, B, NB = 100000, 1000000, 8192, 100
DEV_CAPS = [50, 5, 30, 200, 500, 2000, 100]
D_DEV = 113
D_COMB = 46
D_DEV1, D_DEV2 = 67, 50
D_CH, D_MSG, D_FUS = 27, 67, 56
CAT_IN, D_C1, D_C2 = 106, 63, 31

N_CORES = 8
E_PER = B // N_CORES            # 1024 edges per core
INC_PER = E_PER * NB            # 102400 neighbor incidences per core
G_PER = INC_PER // 128          # 800 gathers per core
CHUNK_E = 32                    # edges per compute chunk
CHUNK_I = CHUNK_E * NB          # 3200 incidences per chunk
G_CHUNK = CHUNK_I // 128        # 25 gathers per chunk
N_CHUNK = E_PER // CHUNK_E      # 32 chunks


def _wrap_clamp_np(i, n):
    """jnp.ndarray[idx] semantics: negative wraps once, then clamp."""
    i = np.where(i < 0, i + n, i)
    return np.clip(i, 0, n - 1)


def _build_xfull(device_feats, tabs):
    """[1M, 128] bf16: col0 cont, cols 1:113 the 7 embeddings in
    reference order (lang, plat, os, country, carrier, brand, plat_os)."""
    out = np.zeros((N_DEV, 128), ml_dtypes.bfloat16)
    out[:, 0] = device_feats[:, 0].astype(ml_dtypes.bfloat16)
    cats = device_feats[:, 1:8].astype(np.int32)
    for i, (cap, t) in enumerate(zip(DEV_CAPS, tabs)):
        c = _wrap_clamp_np(cats[:, i], cap)
        out[:, 1 + 16 * i:17 + 16 * i] = t[c].astype(ml_dtypes.bfloat16)
    return out


def _build_comb_ext(combin_feats, channel_id_emb):
    """[100K, 128] bf16: cols 0:30 cont, 30:46 channel emb, rest 0."""
    out = np.zeros((N_COMBIN, 128), ml_dtypes.bfloat16)
    out[:, :30] = combin_feats[:, :30].astype(ml_dtypes.bfloat16)
    cid = _wrap_clamp_np(combin_feats[:, 30].astype(np.int32), N_COMBIN)
    out[:, 30:46] = channel_id_emb[cid].astype(ml_dtypes.bfloat16)
    return out


def _run(inputs, trace=False):
    import concourse.bass as bass
    import concourse.bacc as bacc
    import concourse.mybir as mybir
    import concourse.tile as tile
    from concourse.bass_utils import run_bass_kernel_spmd

    f32, bf16, i32 = mybir.dt.float32, mybir.dt.bfloat16, mybir.dt.int32

    combin_feats = np.asarray(inputs["combin_feats"], np.float32)
    device_feats = np.asarray(inputs["device_feats"], np.float32)
    channel_id_emb = np.asarray(inputs["channel_id_emb"], np.float32)
    tabs = [np.asarray(inputs[k], np.float32) for k in
            ("lang_emb", "plat_emb", "os_emb", "country_emb",
             "carrier_emb", "brand_emb", "plat_os_emb")]
    edges = np.asarray(inputs["edges"], np.int64)
    neibrs = np.asarray(inputs["sampled_neibrs"], np.int64)

    xfull = _build_xfull(device_feats, tabs)
    comb_ext = _build_comb_ext(combin_feats, channel_id_emb)

    def W(name):
        return np.asarray(inputs[name], np.float32)

    def lhsT_pad(w, kpad):  # [out,in] f32 -> [kpad, out] bf16
        t = np.zeros((kpad, w.shape[0]), np.float32)
        t[: w.shape[1], :] = w.T
        return t.astype(ml_dtypes.bfloat16)

    Wm_l = lhsT_pad(W("W_msg"), 128)                         # [128, 67]
    Wd1_l = lhsT_pad(W("W_dev1"), 128)                       # [128, 67]
    Wch_l = lhsT_pad(W("W_ch1"), 128)                        # [128, 27]
    Wd2_l = lhsT_pad(W("W_dev2"), 67)                        # [67, 50]
    Wfc_l = lhsT_pad(W("W_fus")[:, :D_CH], 27)               # [27, 56]
    Wfm_l = lhsT_pad(W("W_fus")[:, D_CH:] / NB, 67)          # [67, 56]
    Wc1f_l = lhsT_pad(W("W_c1")[:, :D_FUS], 56)              # [56, 63]
    Wc1d_l = lhsT_pad(W("W_c1")[:, D_FUS:], 50)              # [50, 63]
    Wc2_l = lhsT_pad(W("W_c2"), 63)                          # [63, 31]
    Wc3_l = lhsT_pad(W("W_c3"), 31)                          # [31, 1]

    biases = np.zeros((128, 8), np.float32)
    for j, nm in enumerate(("b_msg", "b_dev1", "b_ch1", "b_dev2",
                            "b_fus", "b_c1", "b_c2", "b_c3")):
        b = W(nm)
        biases[: len(b), j] = b

    # ---- host index prep (per core) ----
    e_comb = _wrap_clamp_np(edges[:, 0], N_COMBIN).astype(np.int32)
    e_dev = _wrap_clamp_np(edges[:, 1], N_DEV).astype(np.int32)
    nb_idx = _wrap_clamp_np(neibrs, N_DEV).astype(np.int32)  # [B, 100]

    nbr_idx_np = np.zeros((N_CORES, 128, G_PER), np.int32)
    for c in range(N_CORES):
        flat = nb_idx[c * E_PER:(c + 1) * E_PER].reshape(-1)  # [102400]
        nbr_idx_np[c] = flat.reshape(G_PER, 128).T

    def edge_idx_arr(v):
        out = np.zeros((N_CORES, 128, 8), np.int32)
        for c in range(N_CORES):
            out[c] = v[c * E_PER:(c + 1) * E_PER].reshape(8, 128).T
        return out

    comb_idx_np = edge_idx_arr(e_comb)
    dev_idx_np = edge_idx_arr(e_dev)

    # ---- build bass kernel ----
    nc = bacc.Bacc("TRN2", target_bir_lowering=False, debug=False,
                   num_devices=N_CORES)

    def dram(name, arr, dtype):
        return nc.dram_tensor(name, list(arr.shape), dtype,
                              kind="ExternalInput").ap()

    xf_t = dram("xf_t", xfull, bf16)
    ce_t = dram("ce_t", comb_ext, bf16)
    nbr_t = dram("nbr_t", nbr_idx_np[0], i32)
    ci_t = dram("ci_t", comb_idx_np[0], i32)
    di_t = dram("di_t", dev_idx_np[0], i32)
    wm_t = dram("wm_t", Wm_l, bf16)
    wd1_t = dram("wd1_t", Wd1_l, bf16)
    wch_t = dram("wch_t", Wch_l, bf16)
    wd2_t = dram("wd2_t", Wd2_l, bf16)
    wfc_t = dram("wfc_t", Wfc_l, bf16)
    wfm_t = dram("wfm_t", Wfm_l, bf16)
    wc1f_t = dram("wc1f_t", Wc1f_l, bf16)
    wc1d_t = dram("wc1d_t", Wc1d_l, bf16)
    wc2_t = dram("wc2_t", Wc2_l, bf16)
    wc3_t = dram("wc3_t", Wc3_l, bf16)
    bias_t = dram("bias_t", biases, f32)
    out_t = nc.dram_tensor("out", [1, E_PER], f32, kind="ExternalOutput").ap()

    IOA = bass.IndirectOffsetOnAxis
    AX = mybir.AxisListType
    ALU = mybir.AluOpType
    ACTF = mybir.ActivationFunctionType

    from concourse.masks import make_identity

    with tile.TileContext(nc, trace_sim=False) as tc:
        with tc.tile_pool(name="const", bufs=1) as cpool, \
             tc.tile_pool(name="sbuf", bufs=2) as pool, \
             tc.tile_pool(name="xg", bufs=12) as gpool, \
             tc.tile_pool(name="xt", bufs=3) as xtpool, \
             tc.tile_pool(name="big", bufs=1) as bigpool, \
             tc.tile_pool(name="tp", bufs=4, space="PSUM") as tpp, \
             tc.tile_pool(name="psum", bufs=2, space="PSUM") as pp:

            def cload(name, t, shape, dtype):
                s = cpool.tile(shape, dtype, tag=name)
                nc.sync.dma_start(out=s[:], in_=t[:])
                return s

            # index tiles first so gathers can start immediately; weights
            # stream in behind them on the same sync queue
            di = cload("di", di_t, [128, 8], i32)
            ci = cload("ci", ci_t, [128, 8], i32)
            nbr_i = cload("nbr", nbr_t, [128, G_PER], i32)
            ident = cpool.tile([128, 128], bf16)
            make_identity(nc, ident[:])

            wm = cload("wm", wm_t, [128, 67], bf16)
            wd1 = cload("wd1", wd1_t, [128, 67], bf16)
            wch = cload("wch", wch_t, [128, 27], bf16)
            wd2 = cload("wd2", wd2_t, [67, 50], bf16)
            wfc = cload("wfc", wfc_t, [27, 56], bf16)
            wfm = cload("wfm", wfm_t, [67, 56], bf16)
            wc1f = cload("wc1f", wc1f_t, [56, 63], bf16)
            wc1d = cload("wc1d", wc1d_t, [50, 63], bf16)
            wc2 = cload("wc2", wc2_t, [63, 31], bf16)
            wc3 = cload("wc3", wc3_t, [31, 1], bf16)
            bias = cload("bias", bias_t, [128, 8], f32)

            msg = bigpool.tile([67, E_PER], f32)
            xdt = bigpool.tile([128, E_PER], bf16)
            xct = bigpool.tile([128, E_PER], bf16)

            def gather_T(dst, table, idx_col, k):
                """gather 128 rows of `table` -> PE transpose into dst cols."""
                xg = gpool.tile([128, 128], bf16, tag="xg")
                nc.gpsimd.indirect_dma_start(
                    out=xg[:], out_offset=None, in_=table[:],
                    in_offset=IOA(ap=idx_col, axis=0))
                tp = tpp.tile([128, 128], bf16, tag="tp", space="PSUM")
                nc.tensor.transpose(out=tp[:], in_=xg[:], identity=ident[:])
                if k % 2 == 0:
                    nc.scalar.copy(out=dst, in_=tp[:])
                else:
                    nc.vector.tensor_copy(out=dst, in_=tp[:])

            # ---- edge-branch gathers (16 Pool instrs) ----
            for k in range(8):
                gather_T(xdt[:, k * 128:(k + 1) * 128], xf_t,
                         di[:, k:k + 1], k)
            for k in range(8):
                gather_T(xct[:, k * 128:(k + 1) * 128], ce_t,
                         ci[:, k:k + 1], k)

            # ---- edge MLP part 1: d1, d2, ch ----
            d1 = bigpool.tile([67, E_PER], bf16)
            d2 = bigpool.tile([50, E_PER], bf16)
            ch = bigpool.tile([27, E_PER], bf16)
            for half in range(2):
                sl = slice(half * 512, half * 512 + 512)
                p1 = pp.tile([67, 512], f32, tag="ep", space="PSUM")
                nc.tensor.matmul(out=p1[:], lhsT=wd1[:], rhs=xdt[:, sl],
                                 start=True, stop=True)
                nc.scalar.activation(out=d1[:, sl], in_=p1[:], func=ACTF.Relu,
                                     bias=bias[:67, 1:2], scale=1.0)
                p2 = pp.tile([50, 512], f32, tag="ep", space="PSUM")
                nc.tensor.matmul(out=p2[:], lhsT=wd2[:], rhs=d1[:67, sl],
                                 start=True, stop=True)
                nc.scalar.activation(out=d2[:, sl], in_=p2[:], func=ACTF.Relu,
                                     bias=bias[:50, 3:4], scale=1.0)
                p3 = pp.tile([27, 512], f32, tag="ep", space="PSUM")
                nc.tensor.matmul(out=p3[:], lhsT=wch[:], rhs=xct[:, sl],
                                 start=True, stop=True)
                nc.scalar.activation(out=ch[:, sl], in_=p3[:], func=ACTF.Relu,
                                     bias=bias[:27, 2:3], scale=1.0)

            # ---- neighbor pipeline; head MLP per half as msg completes ----
            msgb = bigpool.tile([67, E_PER], bf16)
            fus = bigpool.tile([56, E_PER], bf16)
            h1 = bigpool.tile([63, E_PER], bf16)
            h2 = bigpool.tile([31, E_PER], bf16)
            hout = bigpool.tile([1, E_PER], f32)

            for c in range(N_CHUNK):
                xt = xtpool.tile([128, CHUNK_I], bf16, tag="xt")
                for t in range(G_CHUNK):
                    g = c * G_CHUNK + t
                    gather_T(xt[:, t * 128:(t + 1) * 128], xf_t,
                             nbr_i[:, g:g + 1], g)
                for k in range(8):
                    p = pp.tile([67, 400], f32, tag="mp", space="PSUM")
                    nc.tensor.matmul(out=p[:], lhsT=wm[:],
                                     rhs=xt[:, k * 400:(k + 1) * 400],
                                     start=True, stop=True)
                    rr = pool.tile([67, 400], bf16, tag="rr")
                    nc.scalar.activation(out=rr[:], in_=p[:], func=ACTF.Relu,
                                         bias=bias[:67, 0:1], scale=1.0)
                    e0 = c * CHUNK_E + k * 4
                    nc.vector.tensor_reduce(
                        out=msg[:, e0:e0 + 4],
                        in_=rr[:].rearrange("p (e k) -> p e k", k=NB),
                        axis=AX.X, op=ALU.add)
                if c % (N_CHUNK // 2) != N_CHUNK // 2 - 1:
                    continue
                # msg for this half of the edges is complete: run the head
                half = c // (N_CHUNK // 2)
                sl = slice(half * 512, half * 512 + 512)
                nc.vector.tensor_copy(out=msgb[:, sl], in_=msg[:, sl])
                p4 = pp.tile([56, 512], f32, tag="ep", space="PSUM")
                nc.tensor.matmul(out=p4[:], lhsT=wfc[:], rhs=ch[:27, sl],
                                 start=True, stop=False)
                nc.tensor.matmul(out=p4[:], lhsT=wfm[:], rhs=msgb[:67, sl],
                                 start=False, stop=True)
                nc.scalar.activation(out=fus[:, sl], in_=p4[:], func=ACTF.Relu,
                                     bias=bias[:56, 4:5], scale=1.0)
                p5 = pp.tile([63, 512], f32, tag="ep", space="PSUM")
                nc.tensor.matmul(out=p5[:], lhsT=wc1f[:], rhs=fus[:56, sl],
                                 start=True, stop=False)
                nc.tensor.matmul(out=p5[:], lhsT=wc1d[:], rhs=d2[:50, sl],
                                 start=False, stop=True)
                nc.scalar.activation(out=h1[:, sl], in_=p5[:], func=ACTF.Relu,
                                     bias=bias[:63, 5:6], scale=1.0)
                p6 = pp.tile([31, 512], f32, tag="ep", space="PSUM")
                nc.tensor.matmul(out=p6[:], lhsT=wc2[:], rhs=h1[:63, sl],
                                 start=True, stop=True)
                nc.scalar.activation(out=h2[:, sl], in_=p6[:], func=ACTF.Relu,
                                     bias=bias[:31, 6:7], scale=1.0)
                p7 = pp.tile([1, 512], f32, tag="ep", space="PSUM")
                nc.tensor.matmul(out=p7[:], lhsT=wc3[:], rhs=h2[:31, sl],
                                 start=True, stop=True)
                nc.scalar.activation(out=hout[:, sl], in_=p7[:],
                                     func=ACTF.Identity, bias=bias[:1, 7:8],
                                     scale=1.0)
                nc.sync.dma_start(out=out_t[:, sl], in_=hout[:, sl])

    nc.compile()

    base = {
        "xf_t": xfull, "ce_t": comb_ext,
        "wm_t": Wm_l, "wd1_t": Wd1_l, "wch_t": Wch_l, "wd2_t": Wd2_l,
        "wfc_t": Wfc_l, "wfm_t": Wfm_l, "wc1f_t": Wc1f_l,
        "wc1d_t": Wc1d_l, "wc2_t": Wc2_l, "wc3_t": Wc3_l, "bias_t": biases,
    }
    in_maps = []
    for c in range(N_CORES):
        m = dict(base)
        m["nbr_t"] = nbr_idx_np[c]
        m["ci_t"] = comb_idx_np[c]
        m["di_t"] = dev_idx_np[c]
        in_maps.append(m)

    res = run_bass_kernel_spmd(nc, in_maps, core_ids=list(range(N_CORES)),
                               trace=trace)
    outs = [res.results[c]["out"].reshape(E_PER) for c in range(N_CORES)]
    full = np.concatenate(outs).reshape(B, 1).astype(np.float32)
    return full, res


def kernel(**inputs):
    out, _ = _run(inputs, trace=False)
    return out


# revision 15
# speedup vs baseline: 4.0428x; 1.0040x over previous
"""BotSpot GNN message-passing kernel for 8 TRN2 NeuronCores (Bass/Tile).

Strategy (data-parallel over the 8192-edge minibatch, 1024 edges/core):
  - host re-encodes the 1M-row device table into a [1M, 128] bf16
    full-feature table (cont + 7 embedding lookups per row) and the
    100K-row combin table into [100K, 128] bf16 (30 cont + channel emb)
  - device gathers one 256B row per neighbor incidence via indirect DMA
    (128 rows / instruction, 800 instructions per core == the SWDGE
    descriptor-generation floor of ~1 descriptor per incidence)
  - XBAR DMA transpose (HWDGE engines) turns gathered [incidence, feat]
    tiles into matmul-ready [feat, incidence] tiles — no PE transposes,
    no PSUM->SBUF copies
  - W_msg matmul + ReLU + positional segmented mean over each edge's 100
    neighbors; small per-edge MLP branches (channel, device, fusion,
    head) on-chip
"""

import numpy as np
import ml_dtypes


def _ensure_ntff_hook_importable():
    """concourse.bass_utils imports antenv.axon_hooks when trace=True (or
    BASS_TRACE is set). Some images lack that module; provide a shim so a
    traced run degrades gracefully instead of raising ImportError."""
    try:
        import antenv.axon_hooks  # noqa: F401
        return
    except Exception:
        pass
    import sys
    import types
    _hook = [None]
    mod = types.ModuleType("antenv.axon_hooks")
    mod.set_axon_ntff_profile_hook = lambda h: _hook.__setitem__(0, h)
    mod.get_axon_ntff_profile_hook = lambda: _hook[0]
    sys.modules["antenv.axon_hooks"] = mod
    try:
        import antenv
        antenv.axon_hooks = mod
    except Exception:
        pass
    try:
        from trn_agent_boot.trn_boot import _ntff_profile_via_ctypes
        mod.set_axon_ntff_profile_hook(
            _ntff_profile_via_ctypes("/opt/axon/libaxon_pjrt.so"))
    except Exception:
        pass


_ensure_ntff_hook_importable()

EMBED = 16
N_COMBIN, N_DEV, B, NB = 100000, 1000000, 8192, 100
DEV_CAPS = [50, 5, 30, 200, 500, 2000, 100]
D_DEV = 113
D_COMB = 46
D_DEV1, D_DEV2 = 67, 50
D_CH, D_MSG, D_FUS = 27, 67, 56
CAT_IN, D_C1, D_C2 = 106, 63, 31

N_CORES = 8
E_PER = B // N_CORES            # 1024 edges per core
INC_PER = E_PER * NB            # 102400 neighbor incidences per core
G_PER = INC_PER // 128          # 800 gathers per core
CHUNK_E = 32                    # edges per compute chunk
CHUNK_I = CHUNK_E * NB          # 3200 incidences per chunk
G_CHUNK = CHUNK_I // 128        # 25 gathers per chunk
N_CHUNK = E_PER // CHUNK_E      # 32 chunks


def _wrap_clamp_np(i, n):
    """jnp.ndarray[idx] semantics: negative wraps once, then clamp."""
    i = np.where(i < 0, i + n, i)
    return np.clip(i, 0, n - 1)


def _build_xfull(device_feats, tabs):
    """[1M, 128] bf16: col0 cont, cols 1:113 the 7 embeddings in
    reference order (lang, plat, os, country, carrier, brand, plat_os)."""
    out = np.zeros((N_DEV, 128), ml_dtypes.bfloat16)
    out[:, 0] = device_feats[:, 0].astype(ml_dtypes.bfloat16)
    cats = device_feats[:, 1:8].astype(np.int32)
    for i, (cap, t) in enumerate(zip(DEV_CAPS, tabs)):
        c = _wrap_clamp_np(cats[:, i], cap)
        out[:, 1 + 16 * i:17 + 16 * i] = t[c].astype(ml_dtypes.bfloat16)
    return out


def _build_comb_ext(combin_feats, channel_id_emb):
    """[100K, 128] bf16: cols 0:30 cont, 30:46 channel emb, rest 0."""
    out = np.zeros((N_COMBIN, 128), ml_dtypes.bfloat16)
    out[:, :30] = combin_feats[:, :30].astype(ml_dtypes.bfloat16)
    cid = _wrap_clamp_np(combin_feats[:, 30].astype(np.int32), N_COMBIN)
    out[:, 30:46] = channel_id_emb[cid].astype(ml_dtypes.bfloat16)
    return out


def _run(inputs, trace=False):
    import concourse.bass as bass
    import concourse.bacc as bacc
    import concourse.mybir as mybir
    import concourse.tile as tile
    from concourse.bass_utils import run_bass_kernel_spmd

    f32, bf16, i32 = mybir.dt.float32, mybir.dt.bfloat16, mybir.dt.int32

    combin_feats = np.asarray(inputs["combin_feats"], np.float32)
    device_feats = np.asarray(inputs["device_feats"], np.float32)
    channel_id_emb = np.asarray(inputs["channel_id_emb"], np.float32)
    tabs = [np.asarray(inputs[k], np.float32) for k in
            ("lang_emb", "plat_emb", "os_emb", "country_emb",
             "carrier_emb", "brand_emb", "plat_os_emb")]
    edges = np.asarray(inputs["edges"], np.int64)
    neibrs = np.asarray(inputs["sampled_neibrs"], np.int64)

    xfull = _build_xfull(device_feats, tabs)
    comb_ext = _build_comb_ext(combin_feats, channel_id_emb)

    def W(name):
        return np.asarray(inputs[name], np.float32)

    def lhsT_pad(w, kpad):  # [out,in] f32 -> [kpad, out] bf16
        t = np.zeros((kpad, w.shape[0]), np.float32)
        t[: w.shape[1], :] = w.T
        return t.astype(ml_dtypes.bfloat16)

    Wm_l = lhsT_pad(W("W_msg"), 128)                         # [128, 67]
    Wd1_l = lhsT_pad(W("W_dev1"), 128)                       # [128, 67]
    Wch_l = lhsT_pad(W("W_ch1"), 128)                        # [128, 27]
    Wd2_l = lhsT_pad(W("W_dev2"), 67)                        # [67, 50]
    Wfc_l = lhsT_pad(W("W_fus")[:, :D_CH], 27)               # [27, 56]
    Wfm_l = lhsT_pad(W("W_fus")[:, D_CH:] / NB, 67)          # [67, 56]
    Wc1f_l = lhsT_pad(W("W_c1")[:, :D_FUS], 56)              # [56, 63]
    Wc1d_l = lhsT_pad(W("W_c1")[:, D_FUS:], 50)              # [50, 63]
    Wc2_l = lhsT_pad(W("W_c2"), 63)                          # [63, 31]
    Wc3_l = lhsT_pad(W("W_c3"), 31)                          # [31, 1]

    biases = np.zeros((128, 8), np.float32)
    for j, nm in enumerate(("b_msg", "b_dev1", "b_ch1", "b_dev2",
                            "b_fus", "b_c1", "b_c2", "b_c3")):
        b = W(nm)
        biases[: len(b), j] = b

    # ---- host index prep (per core) ----
    e_comb = _wrap_clamp_np(edges[:, 0], N_COMBIN).astype(np.int32)
    e_dev = _wrap_clamp_np(edges[:, 1], N_DEV).astype(np.int32)
    nb_idx = _wrap_clamp_np(neibrs, N_DEV).astype(np.int32)  # [B, 100]

    nbr_idx_np = np.zeros((N_CORES, 128, G_PER), np.int32)
    for c in range(N_CORES):
        flat = nb_idx[c * E_PER:(c + 1) * E_PER].reshape(-1)  # [102400]
        nbr_idx_np[c] = flat.reshape(G_PER, 128).T

    def edge_idx_arr(v):
        out = np.zeros((N_CORES, 128, 8), np.int32)
        for c in range(N_CORES):
            out[c] = v[c * E_PER:(c + 1) * E_PER].reshape(8, 128).T
        return out

    comb_idx_np = edge_idx_arr(e_comb)
    dev_idx_np = edge_idx_arr(e_dev)

    # ---- build bass kernel ----
    nc = bacc.Bacc("TRN2", target_bir_lowering=False, debug=False,
                   num_devices=N_CORES)

    def dram(name, arr, dtype):
        return nc.dram_tensor(name, list(arr.shape), dtype,
                              kind="ExternalInput").ap()

    xf_t = dram("xf_t", xfull, bf16)
    ce_t = dram("ce_t", comb_ext, bf16)
    nbr_t = dram("nbr_t", nbr_idx_np[0], i32)
    ci_t = dram("ci_t", comb_idx_np[0], i32)
    di_t = dram("di_t", dev_idx_np[0], i32)
    wm_t = dram("wm_t", Wm_l, bf16)
    wd1_t = dram("wd1_t", Wd1_l, bf16)
    wch_t = dram("wch_t", Wch_l, bf16)
    wd2_t = dram("wd2_t", Wd2_l, bf16)
    wfc_t = dram("wfc_t", Wfc_l, bf16)
    wfm_t = dram("wfm_t", Wfm_l, bf16)
    wc1f_t = dram("wc1f_t", Wc1f_l, bf16)
    wc1d_t = dram("wc1d_t", Wc1d_l, bf16)
    wc2_t = dram("wc2_t", Wc2_l, bf16)
    wc3_t = dram("wc3_t", Wc3_l, bf16)
    bias_t = dram("bias_t", biases, f32)
    out_t = nc.dram_tensor("out", [1, E_PER], f32, kind="ExternalOutput").ap()

    IOA = bass.IndirectOffsetOnAxis
    AX = mybir.AxisListType
    ALU = mybir.AluOpType
    ACTF = mybir.ActivationFunctionType

    from concourse.masks import make_identity

    with tile.TileContext(nc, trace_sim=False) as tc:
        with tc.tile_pool(name="const", bufs=1) as cpool, \
             tc.tile_pool(name="sbuf", bufs=2) as pool, \
             tc.tile_pool(name="xg", bufs=4) as gpool, \
             tc.tile_pool(name="xt", bufs=3) as xtpool, \
             tc.tile_pool(name="big", bufs=1) as bigpool, \
             tc.tile_pool(name="tp", bufs=4, space="PSUM") as tpp, \
             tc.tile_pool(name="psum", bufs=2, space="PSUM") as pp:

            def cload(name, t, shape, dtype):
                s = cpool.tile(shape, dtype, tag=name)
                nc.sync.dma_start(out=s[:], in_=t[:])
                return s

            # index tiles first so gathers can start immediately; weights
            # stream in behind them on the same sync queue
            di = cload("di", di_t, [128, 8], i32)
            ci = cload("ci", ci_t, [128, 8], i32)
            nbr_i = cload("nbr", nbr_t, [128, G_PER], i32)
            ident = cpool.tile([128, 128], bf16)
            make_identity(nc, ident[:])

            wm = cload("wm", wm_t, [128, 67], bf16)
            wd1 = cload("wd1", wd1_t, [128, 67], bf16)
            wch = cload("wch", wch_t, [128, 27], bf16)
            wd2 = cload("wd2", wd2_t, [67, 50], bf16)
            wfc = cload("wfc", wfc_t, [27, 56], bf16)
            wfm = cload("wfm", wfm_t, [67, 56], bf16)
            wc1f = cload("wc1f", wc1f_t, [56, 63], bf16)
            wc1d = cload("wc1d", wc1d_t, [50, 63], bf16)
            wc2 = cload("wc2", wc2_t, [63, 31], bf16)
            wc3 = cload("wc3", wc3_t, [31, 1], bf16)
            bias = cload("bias", bias_t, [128, 8], f32)

            msg = bigpool.tile([67, E_PER], f32)
            xdt = bigpool.tile([128, E_PER], bf16)
            xct = bigpool.tile([128, E_PER], bf16)

            def gather_T(dst, table, idx_col, k, xg=None):
                """gather 128 rows of `table` -> PE transpose into dst cols.

                xg: optional [128, 128] staging slice; sharing one staging
                tile across several gathers batches Pool-side buffer-reuse
                semaphore waits."""
                if xg is None:
                    xge = gpool.tile([128, 128], bf16, tag="xge")
                    xg = xge[:]
                nc.gpsimd.indirect_dma_start(
                    out=xg, out_offset=None, in_=table[:],
                    in_offset=IOA(ap=idx_col, axis=0))
                tp = tpp.tile([128, 128], bf16, tag="tp", space="PSUM")
                nc.tensor.transpose(out=tp[:], in_=xg, identity=ident[:])
                if k % 2 == 0:
                    nc.scalar.copy(out=dst, in_=tp[:])
                else:
                    nc.vector.tensor_copy(out=dst, in_=tp[:])

            # ---- edge-branch gathers (16 Pool instrs) ----
            for k in range(8):
                gather_T(xdt[:, k * 128:(k + 1) * 128], xf_t,
                         di[:, k:k + 1], k)
            for k in range(8):
                gather_T(xct[:, k * 128:(k + 1) * 128], ce_t,
                         ci[:, k:k + 1], k)

            # ---- edge MLP part 1: d1, d2, ch ----
            d1 = bigpool.tile([67, E_PER], bf16)
            d2 = bigpool.tile([50, E_PER], bf16)
            ch = bigpool.tile([27, E_PER], bf16)
            for half in range(2):
                sl = slice(half * 512, half * 512 + 512)
                p1 = pp.tile([67, 512], f32, tag="ep", space="PSUM")
                nc.tensor.matmul(out=p1[:], lhsT=wd1[:], rhs=xdt[:, sl],
                                 start=True, stop=True)
                nc.scalar.activation(out=d1[:, sl], in_=p1[:], func=ACTF.Relu,
                                     bias=bias[:67, 1:2], scale=1.0)
                p2 = pp.tile([50, 512], f32, tag="ep", space="PSUM")
                nc.tensor.matmul(out=p2[:], lhsT=wd2[:], rhs=d1[:67, sl],
                                 start=True, stop=True)
                nc.scalar.activation(out=d2[:, sl], in_=p2[:], func=ACTF.Relu,
                                     bias=bias[:50, 3:4], scale=1.0)
                p3 = pp.tile([27, 512], f32, tag="ep", space="PSUM")
                nc.tensor.matmul(out=p3[:], lhsT=wch[:], rhs=xct[:, sl],
                                 start=True, stop=True)
                nc.scalar.activation(out=ch[:, sl], in_=p3[:], func=ACTF.Relu,
                                     bias=bias[:27, 2:3], scale=1.0)

            # ---- neighbor pipeline; head MLP per half as msg completes ----
            msgb = bigpool.tile([67, E_PER], bf16)
            fus = bigpool.tile([56, E_PER], bf16)
            h1 = bigpool.tile([63, E_PER], bf16)
            h2 = bigpool.tile([31, E_PER], bf16)
            hout = bigpool.tile([1, E_PER], f32)

            for c in range(N_CHUNK):
                xt = xtpool.tile([128, CHUNK_I], bf16, tag="xt")
                for t5 in range(G_CHUNK // 5):
                    xgb = gpool.tile([128, 5 * 128], bf16, tag="xg")
                    for j in range(5):
                        t = t5 * 5 + j
                        g = c * G_CHUNK + t
                        gather_T(xt[:, t * 128:(t + 1) * 128], xf_t,
                                 nbr_i[:, g:g + 1], g,
                                 xg=xgb[:, j * 128:(j + 1) * 128])
                for k in range(8):
                    p = pp.tile([67, 400], f32, tag="mp", space="PSUM")
                    nc.tensor.matmul(out=p[:], lhsT=wm[:],
                                     rhs=xt[:, k * 400:(k + 1) * 400],
                                     start=True, stop=True)
                    rr = pool.tile([67, 400], bf16, tag="rr")
                    nc.scalar.activation(out=rr[:], in_=p[:], func=ACTF.Relu,
                                         bias=bias[:67, 0:1], scale=1.0)
                    e0 = c * CHUNK_E + k * 4
                    nc.vector.tensor_reduce(
                        out=msg[:, e0:e0 + 4],
                        in_=rr[:].rearrange("p (e k) -> p e k", k=NB),
                        axis=AX.X, op=ALU.add)
                if c % (N_CHUNK // 2) != N_CHUNK // 2 - 1:
                    continue
                # msg for this half of the edges is complete: run the head
                half = c // (N_CHUNK // 2)
                sl = slice(half * 512, half * 512 + 512)
                nc.vector.tensor_copy(out=msgb[:, sl], in_=msg[:, sl])
                p4 = pp.tile([56, 512], f32, tag="ep", space="PSUM")
                nc.tensor.matmul(out=p4[:], lhsT=wfc[:], rhs=ch[:27, sl],
                                 start=True, stop=False)
                nc.tensor.matmul(out=p4[:], lhsT=wfm[:], rhs=msgb[:67, sl],
                                 start=False, stop=True)
                nc.scalar.activation(out=fus[:, sl], in_=p4[:], func=ACTF.Relu,
                                     bias=bias[:56, 4:5], scale=1.0)
                p5 = pp.tile([63, 512], f32, tag="ep", space="PSUM")
                nc.tensor.matmul(out=p5[:], lhsT=wc1f[:], rhs=fus[:56, sl],
                                 start=True, stop=False)
                nc.tensor.matmul(out=p5[:], lhsT=wc1d[:], rhs=d2[:50, sl],
                                 start=False, stop=True)
                nc.scalar.activation(out=h1[:, sl], in_=p5[:], func=ACTF.Relu,
                                     bias=bias[:63, 5:6], scale=1.0)
                p6 = pp.tile([31, 512], f32, tag="ep", space="PSUM")
                nc.tensor.matmul(out=p6[:], lhsT=wc2[:], rhs=h1[:63, sl],
                                 start=True, stop=True)
                nc.scalar.activation(out=h2[:, sl], in_=p6[:], func=ACTF.Relu,
                                     bias=bias[:31, 6:7], scale=1.0)
                p7 = pp.tile([1, 512], f32, tag="ep", space="PSUM")
                nc.tensor.matmul(out=p7[:], lhsT=wc3[:], rhs=h2[:31, sl],
                                 start=True, stop=True)
                nc.scalar.activation(out=hout[:, sl], in_=p7[:],
                                     func=ACTF.Identity, bias=bias[:1, 7:8],
                                     scale=1.0)
                nc.sync.dma_start(out=out_t[:, sl], in_=hout[:, sl])

    nc.compile()

    base = {
        "xf_t": xfull, "ce_t": comb_ext,
        "wm_t": Wm_l, "wd1_t": Wd1_l, "wch_t": Wch_l, "wd2_t": Wd2_l,
        "wfc_t": Wfc_l, "wfm_t": Wfm_l, "wc1f_t": Wc1f_l,
        "wc1d_t": Wc1d_l, "wc2_t": Wc2_l, "wc3_t": Wc3_l, "bias_t": biases,
    }
    in_maps = []
    for c in range(N_CORES):
        m = dict(base)
        m["nbr_t"] = nbr_idx_np[c]
        m["ci_t"] = comb_idx_np[c]
        m["di_t"] = dev_idx_np[c]
        in_maps.append(m)

    res = run_bass_kernel_spmd(nc, in_maps, core_ids=list(range(N_CORES)),
                               trace=trace)
    outs = [res.results[c]["out"].reshape(E_PER) for c in range(N_CORES)]
    full = np.concatenate(outs).reshape(B, 1).astype(np.float32)
    return full, res


def kernel(**inputs):
    out, _ = _run(inputs, trace=False)
    return out
